# revision 4
# baseline (speedup 1.0000x reference)
"""Trainium2 Bass kernel for nn_AdaptiveBilinear.

Reference computation (per batch item b, L=2048, D=512):
    a1  = softmax(x1 @ x1^T)        # (L, L)
    a2  = softmax(x2 @ x2^T)        # (L, L)
    x12 = x1 @ x2^T                 # (L, L)
    out = a1 @ x12 @ a2^T           # (L, L)

Key restructure (exact, by matmul associativity):
    out = (a1 @ x1) @ (a2 @ x2)^T = y1 @ y2^T

so each branch is a self-attention with V=X (5*L^2*D FLOPs total instead of
2*L^3 + 3*L^2*D).

Sharding: batch=8 over the 8 NeuronCores, one batch item per core; the
program is pure SPMD with no collectives.

Per-core algorithm (all matmuls bf16 with f32 PSUM accumulation):
    xT = transpose(x)                                 # [D, L], PE transposes
    negdiag[i] = -sum_d x[i,d]^2                      # ones-lhsT matmul over squares
    S'[j,i] = sum_d xT[d,j] xT[d,i] - diag[i]         # K=1 ones-row matmul folds the
                                                      # softmax stabilizer into PSUM
    PT[j,i] = exp(S'[j,i])                            # transposed unnormalized softmax
                                                      # (valid: S symmetric, any per-
                                                      # column constant c[i] is exact)
    sums[i] = sum_j PT[j,i]                           # ones-lhsT matmul
    uT[d,i] = sum_j x[j,d] PT[j,i]                    # natural-layout lhsT; no P transposes
    yT[d,i] = uT[d,i] / sums[i]                       # row-broadcast reciprocal tile
    out[i,l] = sum_d y1T[d,i] y2T[d,l]
"""

import numpy as np

import concourse.bass as bass
import concourse.mybir as mybir
import concourse.tile as tile
from concourse import bacc, bass_utils
from concourse.masks import make_identity

F32 = mybir.dt.float32
BF16 = mybir.dt.bfloat16
EXP = mybir.ActivationFunctionType.Exp

L = 2048          # sequence length per batch item
D = 512           # feature dim
NB = L // 128     # 16 row blocks
DC = D // 128     # 4 contraction chunks of 128
NC = L // 512     # 4 moving-free chunks of 512
N_CORES = 8


def _build_branch(nc, tc, bi, sb_pools, x_d, yT, consts):
    """Process one attention branch: x (DRAM) -> yT [128, DC, L] bf16 (SBUF)."""
    ident, ones_col, ones_row, ones_row_f32 = consts
    stage, work = sb_pools

    with tc.tile_pool(name=f"branch{bi}", bufs=1) as bp:
        xb = bp.tile([128, NB, D], BF16, tag="xb")
        xT = bp.tile([128, DC, L], BF16, tag="xT")
        PT = bp.tile([128, NB, L], BF16, tag="PT")
        RS = bp.tile([128, L], F32, tag="RS")
        negdiag = bp.tile([1, L], BF16, tag="negdiag")
        rsum_row = bp.tile([1, L], F32, tag="rsum_row")

        # --- load + cast + transpose; negdiag[i] = -sum_d x[i,d]^2 ---
        with (
            tc.tile_pool(name=f"ps_tp{bi}", bufs=3, space="PSUM") as ps_tp,
            tc.tile_pool(name=f"ps_nd{bi}", bufs=1, space="PSUM") as ps_nd,
        ):
            for j in range(NB):
                stg = stage.tile([128, D], F32, tag="stg")
                nc.sync.dma_start(stg[:], x_d.ap()[j * 128:(j + 1) * 128, :])
                nc.vector.tensor_copy(xb[:, j, :], stg[:])
                for c in range(DC):
                    tp = ps_tp.tile([128, 128], BF16, tag="tp")
                    nc.tensor.transpose(
                        tp[:], xb[:, j, c * 128:(c + 1) * 128], ident[:])
                    nc.any.tensor_copy(xT[:, c, j * 128:(j + 1) * 128], tp[:])

            nd_ps = ps_nd.tile([1, L], F32, tag="nd")
            for c in range(DC):
                sq = work.tile([128, L], BF16, tag="sq")
                nc.vector.tensor_mul(sq[:], xT[:, c, :], xT[:, c, :])
                for n in range(NC):
                    nc.tensor.matmul(
                        nd_ps[:, n * 512:(n + 1) * 512],
                        ones_col[:],
                        sq[:, n * 512:(n + 1) * 512],
                        start=(c == 0),
                        stop=(c == DC - 1),
                    )
            nc.vector.tensor_scalar_mul(negdiag[:], nd_ps[:], -1.0)

        # --- S'[j,:] = x_j x^T - diag ; PT[j,:] = exp(S') ---
        with tc.tile_pool(name=f"ps_s{bi}", bufs=2, space="PSUM") as ps_s:
            for j in range(NB):
                sps = ps_s.tile([128, L], F32, tag="S")
                for c in range(DC):
                    for n in range(NC):
                        nc.tensor.matmul(
                            sps[:, n * 512:(n + 1) * 512],
                            xT[:, c, j * 128:(j + 1) * 128],
                            xT[:, c, n * 512:(n + 1) * 512],
                            start=(c == 0),
                            stop=False,
                        )
                for n in range(NC):
                    nc.tensor.matmul(
                        sps[:, n * 512:(n + 1) * 512],
                        ones_row[:],
                        negdiag[:, n * 512:(n + 1) * 512],
                        start=False,
                        stop=True,
                    )
                nc.scalar.activation(PT[:, j, :], sps[:], EXP)

        # --- column sums of PT (= softmax row sums), reciprocal, broadcast ---
        with (
            tc.tile_pool(name=f"ps_sum{bi}", bufs=1, space="PSUM") as ps_sum,
            tc.tile_pool(name=f"ps_rs{bi}", bufs=2, space="PSUM") as ps_rs,
        ):
            sums_ps = ps_sum.tile([1, L], F32, tag="sums")
            for j in range(NB):
                for n in range(NC):
                    nc.tensor.matmul(
                        sums_ps[:, n * 512:(n + 1) * 512],
                        ones_col[:],
                        PT[:, j, n * 512:(n + 1) * 512],
                        start=(j == 0),
                        stop=(j == NB - 1),
                    )
            nc.vector.reciprocal(rsum_row[:], sums_ps[:])
            for n in range(NC):
                rsb = ps_rs.tile([128, 512], F32, tag="rsb")
                nc.tensor.matmul(
                    rsb[:],
                    ones_row_f32[:],
                    rsum_row[:, n * 512:(n + 1) * 512],
                    start=True,
                    stop=True,
                )
                nc.any.tensor_copy(RS[:, n * 512:(n + 1) * 512], rsb[:])

        # --- uT[d,i] = sum_j x[j,d] PT[j,i]; yT = uT * RS ---
        with tc.tile_pool(name=f"ps_u{bi}", bufs=8, space="PSUM") as ps_u:
            for c in range(DC):
                ups = [ps_u.tile([128, 512], F32, tag="u", name=f"u{bi}_{c}_{n}")
                       for n in range(NC)]
                for j in range(NB):
                    for n in range(NC):
                        nc.tensor.matmul(
                            ups[n][:],
                            xb[:, j, c * 128:(c + 1) * 128],
                            PT[:, j, n * 512:(n + 1) * 512],
                            start=(j == 0),
                            stop=(j == NB - 1),
                        )
                for n in range(NC):
                    nc.vector.tensor_mul(
                        yT[:, c, n * 512:(n + 1) * 512],
                        ups[n][:],
                        RS[:, n * 512:(n + 1) * 512],
                    )


def build_nc():
    nc = bacc.Bacc("TRN2", target_bir_lowering=False, debug=False,
                   num_devices=N_CORES)
    x1_d = nc.dram_tensor("x1", [L, D], F32, kind="ExternalInput")
    x2_d = nc.dram_tensor("x2", [L, D], F32, kind="ExternalInput")
    out_d = nc.dram_tensor("out", [L, L], F32, kind="ExternalOutput")

    with tile.TileContext(nc) as tc:
        with (
            tc.tile_pool(name="const", bufs=1) as constp,
            tc.tile_pool(name="persist", bufs=1) as pp,
            tc.tile_pool(name="stage", bufs=4) as stage,
            tc.tile_pool(name="work", bufs=2) as work,
        ):
            ident = constp.tile([128, 128], BF16, tag="ident")
            make_identity(nc, ident[:])
            ones_col = constp.tile([128, 1], BF16, tag="ones_col")
            nc.gpsimd.memset(ones_col[:], 1.0)
            ones_row = constp.tile([1, 128], BF16, tag="ones_row")
            nc.gpsimd.memset(ones_row[:], 1.0)
            ones_row_f32 = constp.tile([1, 128], F32, tag="ones_row_f32")
            nc.gpsimd.memset(ones_row_f32[:], 1.0)
            consts = (ident, ones_col, ones_row, ones_row_f32)

            y1T = pp.tile([128, DC, L], BF16, tag="y1T")
            y2T = pp.tile([128, DC, L], BF16, tag="y2T")

            sb_pools = (stage, work)
            _build_branch(nc, tc, 1, sb_pools, x1_d, y1T, consts)
            _build_branch(nc, tc, 2, sb_pools, x2_d, y2T, consts)

            # --- out[i,l] = sum_d y1T[d,i] y2T[d,l] ---
            with tc.tile_pool(name="ps_o", bufs=2, space="PSUM") as ps_o:
                for i in range(NB):
                    ops = ps_o.tile([128, L], F32, tag="o")
                    for c in range(DC):
                        for n in range(NC):
                            nc.tensor.matmul(
                                ops[:, n * 512:(n + 1) * 512],
                                y1T[:, c, i * 128:(i + 1) * 128],
                                y2T[:, c, n * 512:(n + 1) * 512],
                                start=(c == 0),
                                stop=(c == DC - 1),
                            )
                    osb = stage.tile([128, L], F32, tag="osb")
                    nc.any.tensor_copy(osb[:], ops[:])
                    nc.sync.dma_start(out_d.ap()[i * 128:(i + 1) * 128, :], osb[:])

    nc.compile()
    return nc


_NC_CACHE = None


def _get_nc():
    global _NC_CACHE
    if _NC_CACHE is None:
        _NC_CACHE = build_nc()
    return _NC_CACHE


def kernel(x1: np.ndarray, x2: np.ndarray) -> np.ndarray:
    """Full inputs (8, 2048, 512) f32 -> full output (8, 2048, 2048) f32."""
    assert x1.shape == (N_CORES, L, D) and x2.shape == (N_CORES, L, D)
    nc = _get_nc()
    in_maps = [
        {
            "x1": np.ascontiguousarray(np.asarray(x1[b], dtype=np.float32)),
            "x2": np.ascontiguousarray(np.asarray(x2[b], dtype=np.float32)),
        }
        for b in range(N_CORES)
    ]
    res = bass_utils.run_bass_kernel_spmd(nc, in_maps, core_ids=list(range(N_CORES)))
    out = np.stack([res.results[b]["out"] for b in range(N_CORES)], axis=0)
    return out.astype(np.float32, copy=False)


if __name__ == "__main__":
    rng = np.random.default_rng(0)
    x1 = rng.standard_normal((N_CORES, L, D), dtype=np.float32)
    x2 = rng.standard_normal((N_CORES, L, D), dtype=np.float32)
    out = kernel(x1=x1, x2=x2)
    print("kernel output:", out.shape, out.dtype)
